# revision 3
# baseline (speedup 1.0000x reference)
"""Nystrom attention TRN2 kernel, v2.

Key changes vs baseline:
- Q/K loaded fp32 via HWDGE (no SWDGE cast); transposes run in fp32r
  (1.5 cyc/row); PSUM->SBUF copies cast to bf16.
- Landmark pooling via DVE/Pool tensor_reduce over Q^T/K^T columns
  (eliminates 64 PE matmuls + ldweights per pair).
- 2 pairs processed per "superpair" with partition stacking [128 = 2x64]:
  all small vector/scalar ops (k2 softmax, NS smalls, CV norm) run once
  per superpair at full 128-partition width; E1 logits/exp and X' use
  block-diagonal stationaries to halve matmul count.
- X' computed stacked via block-diag M2' [128, 130]; normalize grouped.
- V loaded bf16 via SWDGE (cast in DMA) as in baseline.
"""

import math

import numpy as np

import concourse.bass as bass
import concourse.tile as tile
from concourse import bacc, mybir

F32 = mybir.dt.float32
F32R = mybir.dt.float32r
BF16 = mybir.dt.bfloat16

B, H, S, D = 4, 16, 4096, 64
M = 64
SEG = S // M          # 64
NT = S // 128         # 32
N_CORES = 8
PAIRS = (B * H) // N_CORES   # 8 pairs per core
NSP = PAIRS // 2             # 4 superpairs
SCALE2 = 1.0 / math.sqrt(D)  # s^2
C1 = SCALE2 / SEG            # s^2/64 (scaled pooling)
CRAW = 1.0 / SEG             # raw mean pooling

Exp = mybir.ActivationFunctionType.Exp
Alu = mybir.AluOpType
AX = mybir.AxisListType


def _consts():
    i128 = np.eye(128, dtype=np.float32)
    ibd2 = np.concatenate([np.eye(64), np.eye(64)], 0).astype(np.float32)  # [128,64]
    ones128 = np.ones((128, 1), dtype=np.float32)
    sel2 = np.zeros((2, 128), dtype=np.float32)
    sel2[0, 0:64] = 1.0
    sel2[1, 64:128] = 1.0
    par2 = np.zeros((128, 8), dtype=np.float32)
    for a in range(8):
        par2[16 * a:16 * (a + 1), a] = 1.0
    selt = sel2.T.copy()
    return i128, ibd2, ones128, sel2, par2, selt


def build_body(tc, ctx, q_d, k_d, v_d, x_d, n_pairs):
    nc = tc.nc
    i128_np, ibd2_np, ones128_np, sel2_np, par2_np, selt_np = _consts()

    i128b_dram = nc.inline_tensor(i128_np.astype(mybir.dt.np(BF16)), name="i128b")
    ibd2_dram = nc.inline_tensor(ibd2_np.astype(mybir.dt.np(BF16)), name="ibd2")
    iq2_dram = nc.inline_tensor(
        (3.25 * ibd2_np).astype(mybir.dt.np(BF16)), name="iq2"
    )
    ones_dram = nc.inline_tensor(ones128_np.astype(mybir.dt.np(BF16)), name="ones128")
    sel2_dram = nc.inline_tensor(sel2_np.astype(mybir.dt.np(BF16)), name="sel2")
    par2_dram = nc.inline_tensor(par2_np.astype(mybir.dt.np(BF16)), name="par2")
    selt_dram = nc.inline_tensor(selt_np.astype(mybir.dt.np(BF16)), name="selt")

    cpool = ctx.enter_context(tc.tile_pool(name="consts", bufs=1))
    qkpool = ctx.enter_context(tc.tile_pool(name="qk", bufs=3))
    vpool = ctx.enter_context(tc.tile_pool(name="v", bufs=3))
    tpool = ctx.enter_context(tc.tile_pool(name="trans", bufs=2))
    epool = ctx.enter_context(tc.tile_pool(name="exps", bufs=3))
    opool = ctx.enter_context(tc.tile_pool(name="outs", bufs=2))
    spool = ctx.enter_context(tc.tile_pool(name="smalls", bufs=4))
    ps_big = ctx.enter_context(tc.tile_pool(name="ps_big", bufs=2, space="PSUM"))
    ps_acc = ctx.enter_context(tc.tile_pool(name="ps_acc", bufs=1, space="PSUM"))
    ps_sm = ctx.enter_context(tc.tile_pool(name="ps_sm", bufs=2, space="PSUM"))

    i128b = cpool.tile([128, 128], BF16)
    nc.sync.dma_start(out=i128b[:], in_=i128b_dram[:])
    ibd2 = cpool.tile([128, 64], BF16)
    nc.sync.dma_start(out=ibd2[:], in_=ibd2_dram[:])
    iq2 = cpool.tile([128, 64], BF16)
    nc.sync.dma_start(out=iq2[:], in_=iq2_dram[:])
    ones128 = cpool.tile([128, 1], BF16)
    nc.sync.dma_start(out=ones128[:], in_=ones_dram[:])
    sel2 = cpool.tile([2, 128], BF16)
    nc.sync.dma_start(out=sel2[:], in_=sel2_dram[:])
    par16 = cpool.tile([128, 8], BF16)
    nc.sync.dma_start(out=par16[:], in_=par2_dram[:])
    selt = cpool.tile([128, 2], BF16)
    nc.sync.dma_start(out=selt[:], in_=selt_dram[:])

    for u in range(n_pairs // 2):
        pa, pb = 2 * u, 2 * u + 1

        # ---- loads ----
        qn = {}
        kn = {}
        for p, half in ((pa, 0), (pb, 1)):
            qn[half] = qkpool.tile([128, NT // 4, 4, 64], BF16, tag=f"qn{half}", name=f"qn{half}_{u}")
            nc.gpsimd.dma_start(
                out=qn[half][:].rearrange("p t q d -> p t (q d)"),
                in_=q_d[p].rearrange("(t p q) d -> p t (q d)", p=128, q=4),
            )
            kn[half] = qkpool.tile([128, NT // 4, 4, 64], BF16, tag=f"kn{half}", name=f"kn{half}_{u}")
            nc.gpsimd.dma_start(
                out=kn[half][:].rearrange("p t q d -> p t (q d)"),
                in_=k_d[p].rearrange("(t p q) d -> p t (q d)", p=128, q=4),
            )
        vb = {}
        for p, half in ((pa, 0), (pb, 1)):
            vb[half] = vpool.tile([128, NT // 4, 256], BF16, tag=f"vb{half}", name=f"vb{half}_{u}")
            nc.gpsimd.dma_start(
                out=vb[half][:],
                in_=v_d[p].rearrange("(t p q) d -> p t (q d)", p=128, q=4),
            )

        # ---- landmark pooling via column-window reduce ----
        qlm_ps = ps_acc.tile([128, 64], F32, tag="acc2")
        klm_ps = ps_acc.tile([128, 64], F32, tag="acc")
        for half in (0, 1):
            h = 64 * half
            for t in range(NT // 4):
                for j in range(4):
                    nc.tensor.matmul(
                        qlm_ps[h:h + 64, 8 * t:8 * t + 8],
                        qn[half][:, t, j, :],
                        par16[:],
                        start=(j == 0),
                        stop=(j == 3),
                    )
                    nc.tensor.matmul(
                        klm_ps[h:h + 64, 8 * t:8 * t + 8],
                        kn[half][:, t, j, :],
                        par16[:],
                        start=(j == 0),
                        stop=(j == 3),
                    )
        qlm2 = spool.tile([128, 64], BF16, tag="qlm2")
        nc.vector.tensor_copy(qlm2[:], qlm_ps[:])
        klm2 = spool.tile([128, 64], BF16, tag="klm2")
        nc.vector.tensor_copy(klm2[:], klm_ps[:])

        # ---- small scaled forms ----
        klmRAW = spool.tile([128, 64], BF16, tag="klmRAW")
        nc.vector.tensor_scalar_mul(klmRAW[:], klm2[:], CRAW)
        klmBD = spool.tile([128, 128], BF16, tag="klmBD")
        nc.vector.memset(klmBD[:], 0.0)
        nc.vector.tensor_scalar_mul(klmBD[0:64, 0:64], klm2[0:64, :], C1)
        nc.vector.tensor_scalar_mul(klmBD[64:128, 64:128], klm2[64:128, :], C1)
        qlmBD = spool.tile([128, 128], BF16, tag="qlmBD")
        nc.vector.memset(qlmBD[:], 0.0)
        nc.vector.tensor_scalar_mul(qlmBD[0:64, 0:64], qlm2[0:64, :], C1)
        nc.vector.tensor_scalar_mul(qlmBD[64:128, 64:128], qlm2[64:128, :], C1)

        # ---- kernel_2 = rownorm(exp(qlmBD.T @ klmRAW)), stacked [128, 64] ----
        l2_ps = ps_sm.tile([128, 64], F32, tag="sm")
        nc.tensor.matmul(l2_ps[:], qlmBD[:], klmRAW[:])
        e2 = spool.tile([128, 64], F32, tag="e2")
        d2 = spool.tile([128, 1], F32, tag="d2")
        nc.scalar.activation(e2[:], l2_ps[:], Exp, accum_out=d2[:])
        d2i = spool.tile([128, 1], F32, tag="d2i")
        nc.vector.reciprocal(d2i[:], d2[:])
        k2s = spool.tile([128, 64], BF16, tag="k2s")
        nc.vector.tensor_scalar_mul(k2s[:], e2[:], d2i[:])

        # k2^T per pair -> stacked [128, 64]
        k2t_ps = ps_sm.tile([128, 64], BF16, tag="sm")
        nc.tensor.transpose(k2t_ps[0:64, :], k2s[0:64, :], i128b[0:64, 0:64])
        nc.tensor.transpose(
            k2t_ps[64:128, :], k2s[64:128, :], i128b[64:128, 64:128]
        )
        k2ts = spool.tile([128, 64], BF16, tag="k2ts")
        nc.vector.tensor_copy(k2ts[:], k2t_ps[:])

        # ---- NS init: per-pair scale = 1/max(colsum) ----
        crow_ps = ps_sm.tile([2, 64], F32, tag="sm")
        nc.tensor.matmul(crow_ps[:], selt[:], k2s[:])
        mx = spool.tile([2, 1], F32, tag="mx")
        nc.vector.tensor_reduce(mx[:], crow_ps[:], axis=AX.X, op=Alu.max)
        scif = spool.tile([2, 1], F32, tag="scif")
        nc.vector.reciprocal(scif[:], mx[:])
        scit = spool.tile([2, 1], BF16, tag="scit")
        nc.vector.tensor_copy(scit[:], scif[:])
        scb_ps = ps_sm.tile([128, 1], F32, tag="sm")
        nc.tensor.matmul(scb_ps[:], sel2[:], scit[:])
        scb = spool.tile([128, 1], F32, tag="scb")
        nc.vector.tensor_copy(scb[:], scb_ps[:])

        vc = spool.tile([128, 64], BF16, tag="vc0")
        nc.vector.tensor_scalar_mul(vc[:], k2ts[:], scb[:])
        vct = spool.tile([128, 64], BF16, tag="vct0")
        nc.vector.tensor_scalar_mul(vct[:], k2s[:], scb[:])

        # ---- 6 Newton-Schulz iterations, per-pair matmuls + stacked smalls ----
        for i in range(6):
            a_ps = ps_sm.tile([128, 64], F32, tag="sm")
            at_ps = ps_sm.tile([128, 64], F32, tag="sm")
            for h in (0, 64):
                sl = slice(h, h + 64)
                nc.tensor.matmul(a_ps[sl, :], k2ts[sl, :], vc[sl, :])
                nc.tensor.matmul(at_ps[sl, :], vc[sl, :], k2ts[sl, :])
            at_sb = spool.tile([128, 64], BF16, tag="at_sb")
            nc.scalar.copy(at_sb[:], at_ps[:])
            b_sb = spool.tile([128, 64], BF16, tag="b_sb")
            nc.vector.scalar_tensor_tensor(
                b_sb[:], ibd2[:], 7.0, a_ps[:], op0=Alu.mult, op1=Alu.subtract
            )
            cc_ps = ps_sm.tile([128, 64], F32, tag="sm")
            for h in (0, 64):
                sl = slice(h, h + 64)
                nc.tensor.matmul(cc_ps[sl, :], at_sb[sl, :], b_sb[sl, :])
            d_sb = spool.tile([128, 64], BF16, tag="d_sb")
            nc.vector.scalar_tensor_tensor(
                d_sb[:], ibd2[:], 15.0, cc_ps[:], op0=Alu.mult, op1=Alu.subtract
            )
            f_ps = ps_sm.tile([128, 64], F32, tag="sm")
            for h in (0, 64):
                sl = slice(h, h + 64)
                nc.tensor.matmul(f_ps[sl, :], at_sb[sl, :], d_sb[sl, :])
            g_sb = spool.tile([128, 64], BF16, tag="g_sb")
            nc.vector.scalar_tensor_tensor(
                g_sb[:], f_ps[:], -0.25, iq2[:], op0=Alu.mult, op1=Alu.add
            )
            vnt_ps = ps_sm.tile([128, 64], F32, tag="sm")
            for h in (0, 64):
                sl = slice(h, h + 64)
                nc.tensor.matmul(vnt_ps[sl, :], g_sb[sl, :], vct[sl, :])
            if i < 5:
                vn_ps = ps_sm.tile([128, 64], F32, tag="sm")
                for h in (0, 64):
                    sl = slice(h, h + 64)
                    nc.tensor.matmul(vn_ps[sl, :], vct[sl, :], g_sb[sl, :])
                vc = spool.tile([128, 64], BF16, tag="vc", name=f"vc_{u}_{i}")
                nc.scalar.copy(vc[:], vn_ps[:])
            vct = spool.tile([128, 64], BF16, tag="vct", name=f"vct_{u}_{i}")
            nc.vector.tensor_copy(vct[:], vnt_ps[:])

        # ---- transposes: qt2/kt2 [128 = 2x64 d, S] bf16 ----
        qt2 = tpool.tile([128, S], BF16, tag="qt2")
        kt2 = tpool.tile([128, S], BF16, tag="kt2")
        for src, dst, ceng in ((qn, qt2, "act"), (kn, kt2, "dve")):
            for g in range(8):
                tp = ps_big.tile([128, 512], BF16, tag="bigb")
                for j in range(4):
                    t = 4 * g + j
                    for half in (0, 1):
                        nc.tensor.transpose(
                            tp[64 * half:64 * half + 64, 128 * j:128 * (j + 1)],
                            src[half][:, t // 4, t % 4, :],
                            i128b[:],
                        )
                if ceng == "act":
                    nc.scalar.copy(dst[:, 512 * g:512 * (g + 1)], tp[:])
                else:
                    nc.vector.tensor_copy(dst[:, 512 * g:512 * (g + 1)], tp[:])

        # ---- E1 stacked: e1t2 [128 = 2x64 m, S] ----
        e1t2 = epool.tile([128, S], BF16, tag="e1t2")
        for w in range(8):
            l1_ps = ps_big.tile([128, 512], F32, tag="big")
            nc.tensor.matmul(l1_ps[:], klmBD[:], qt2[:, 512 * w:512 * (w + 1)])
            nc.scalar.activation(e1t2[:, 512 * w:512 * (w + 1)], l1_ps[:], Exp)

        # ---- E3 + CV ----
        e3t = epool.tile([128, NT * 128], BF16, tag="e3t")
        for g in range(8):
            l3_ps = ps_big.tile([128, 512], F32, tag="big")
            for j in range(4):
                w = 4 * g + j
                nc.tensor.matmul(
                    l3_ps[:, 128 * j:128 * (j + 1)],
                    kt2[:, 128 * w:128 * (w + 1)],
                    qlmBD[:],
                )
            nc.scalar.activation(e3t[:, 512 * g:512 * (g + 1)], l3_ps[:], Exp)
        cv_ps = ps_acc.tile([128, 64], F32, tag="acc2")
        cvd_ps = ps_acc.tile([128, 1], F32, tag="acc")
        for half in (0, 1):
            h = 64 * half
            for t in range(NT):
                sl3 = e3t[:, 128 * t + 64 * half:128 * t + 64 * half + 64]
                nc.tensor.matmul(
                    cv_ps[h:h + 64, :],
                    sl3,
                    vb[half][:, t // 4, 64 * (t % 4):64 * (t % 4) + 64],
                    start=(t == 0),
                    stop=(t == NT - 1),
                )
                nc.tensor.matmul(
                    cvd_ps[h:h + 64, :],
                    sl3,
                    ones128[:],
                    start=(t == 0),
                    stop=(t == NT - 1),
                )
        d3i = spool.tile([128, 1], F32, tag="d3i")
        nc.vector.reciprocal(d3i[:], cvd_ps[:])
        cv_sb = spool.tile([128, 64], BF16, tag="cv_sb")
        nc.vector.tensor_scalar_mul(cv_sb[:], cv_ps[:], d3i[:])

        # ---- M2' block-diag [128, 130] ----
        m2_ps = ps_sm.tile([128, 64], F32, tag="sm")
        for h in (0, 64):
            sl = slice(h, h + 64)
            nc.tensor.matmul(m2_ps[sl, :], vct[sl, :], cv_sb[sl, :])
        m2bd = spool.tile([128, 130], BF16, tag="m2bd")
        nc.vector.memset(m2bd[:], 0.0)
        nc.scalar.copy(m2bd[0:64, 0:64], m2_ps[0:64, :])
        nc.scalar.copy(m2bd[64:128, 65:129], m2_ps[64:128, :])
        nc.vector.memset(m2bd[0:64, 64:65], 1.0)
        nc.vector.memset(m2bd[64:128, 129:130], 1.0)

        # ---- X' stacked via m2bd; normalize; store ----
        xsb = opool.tile([128, NT // 4, 2, 4, 64], F32, tag="xsb")
        for T in range(NT // 4):
            for qh in range(2):
                xpt = ps_big.tile([128, 512], F32, tag="big")
                xp = xpt[:, 0:260].rearrange("p (c e) -> p c e", e=130)
                for j in range(2):
                    w = 4 * T + 2 * qh + j
                    nc.tensor.matmul(
                        xp[:, j, :], e1t2[:, 128 * w:128 * (w + 1)], m2bd[:]
                    )
                xp4 = xp.rearrange("p c (q e) -> p c q e", q=2)
                dgi = spool.tile([128, 2, 2], F32, tag="dgi")
                nc.vector.reciprocal(dgi[:], xp4[:, :, :, 64])
                nc.vector.tensor_tensor(
                    xsb[:, T, :, 2 * qh:2 * qh + 2, :].rearrange(
                        "p r j d -> p j r d"
                    ),
                    xp4[:, :, :, 0:64],
                    dgi[:].rearrange("p c (q b) -> p c q b", b=1)
                    .broadcast_to([128, 2, 2, 64]),
                    op=Alu.mult,
                )
            if T in (3, NT // 4 - 1):
                lo = 0 if T == 3 else 4
                for p, half in ((pa, 0), (pb, 1)):
                    nc.sync.dma_start(
                        out=x_d[p].rearrange(
                            "(t p q) d -> p t (q d)", p=128, q=4
                        )[:, lo:T + 1, :],
                        in_=xsb[:, lo:T + 1, half, :, :].rearrange(
                            "p t q d -> p t (q d)"
                        ),
                    )


def build_nc(n_pairs=PAIRS, reps=1):
    from contextlib import ExitStack

    nc = bacc.Bacc("TRN2", target_bir_lowering=False, debug=False)
    q_d = nc.declare_dram_parameter("Q", [n_pairs, S, D], F32, isOutput=False)
    k_d = nc.declare_dram_parameter("K", [n_pairs, S, D], F32, isOutput=False)
    v_d = nc.declare_dram_parameter("V", [n_pairs, S, D], F32, isOutput=False)
    x_d = nc.declare_dram_parameter("X", [n_pairs, S, D], F32, isOutput=True)
    with tile.TileContext(nc) as tc:
        with ExitStack() as ctx:
            if reps == 1:
                build_body(tc, ctx, q_d[:], k_d[:], v_d[:], x_d[:], n_pairs)
            else:
                with tc.For_i(0, reps, 1):
                    build_body(tc, ctx, q_d[:], k_d[:], v_d[:], x_d[:], n_pairs)
    nc.finalize()
    return nc


_CACHED = {}


def kernel(Q: np.ndarray, K: np.ndarray, V: np.ndarray) -> np.ndarray:
    from concourse.bass_utils import run_bass_kernel_spmd

    if "nc" not in _CACHED:
        _CACHED["nc"] = build_nc()
    nc = _CACHED["nc"]

    qf = np.ascontiguousarray(Q.reshape(B * H, S, D), dtype=np.float32)
    kf = np.ascontiguousarray(K.reshape(B * H, S, D), dtype=np.float32)
    vf = np.ascontiguousarray(V.reshape(B * H, S, D), dtype=np.float32)
    core_ids = list(range(N_CORES))
    in_maps = [
        {
            "Q": qf[c * PAIRS:(c + 1) * PAIRS],
            "K": kf[c * PAIRS:(c + 1) * PAIRS],
            "V": vf[c * PAIRS:(c + 1) * PAIRS],
        }
        for c in core_ids
    ]
    res = run_bass_kernel_spmd(nc, in_maps, core_ids)
    out = np.concatenate([res.results[c]["X"] for c in core_ids], axis=0)
    return out.reshape(B, H, S, D)


# revision 4
# speedup vs baseline: 1.0829x; 1.0829x over previous
"""Nystrom attention TRN2 kernel, v2.

Key changes vs baseline:
- Q/K loaded fp32 via HWDGE (no SWDGE cast); transposes run in fp32r
  (1.5 cyc/row); PSUM->SBUF copies cast to bf16.
- Landmark pooling via DVE/Pool tensor_reduce over Q^T/K^T columns
  (eliminates 64 PE matmuls + ldweights per pair).
- 2 pairs processed per "superpair" with partition stacking [128 = 2x64]:
  all small vector/scalar ops (k2 softmax, NS smalls, CV norm) run once
  per superpair at full 128-partition width; E1 logits/exp and X' use
  block-diagonal stationaries to halve matmul count.
- X' computed stacked via block-diag M2' [128, 130]; normalize grouped.
- V loaded bf16 via SWDGE (cast in DMA) as in baseline.
"""

import math

import numpy as np

import concourse.bass as bass
import concourse.tile as tile
from concourse import bacc, mybir

F32 = mybir.dt.float32
F32R = mybir.dt.float32r
BF16 = mybir.dt.bfloat16

B, H, S, D = 4, 16, 4096, 64
M = 64
SEG = S // M          # 64
NT = S // 128         # 32
N_CORES = 8
PAIRS = (B * H) // N_CORES   # 8 pairs per core
NSP = PAIRS // 2             # 4 superpairs
SCALE2 = 1.0 / math.sqrt(D)  # s^2
C1 = SCALE2 / SEG            # s^2/64 (scaled pooling)
CRAW = 1.0 / SEG             # raw mean pooling

Exp = mybir.ActivationFunctionType.Exp
Alu = mybir.AluOpType
AX = mybir.AxisListType


def _consts():
    i128 = np.eye(128, dtype=np.float32)
    ibd2 = np.concatenate([np.eye(64), np.eye(64)], 0).astype(np.float32)  # [128,64]
    ones128 = np.ones((128, 1), dtype=np.float32)
    sel2 = np.zeros((2, 128), dtype=np.float32)
    sel2[0, 0:64] = 1.0
    sel2[1, 64:128] = 1.0
    par2 = np.zeros((128, 8), dtype=np.float32)
    for a in range(8):
        par2[16 * a:16 * (a + 1), a] = 1.0
    selt = sel2.T.copy()
    return i128, ibd2, ones128, sel2, par2, selt


def build_body(tc, ctx, q_d, k_d, v_d, x_d, n_pairs):
    nc = tc.nc
    i128_np, ibd2_np, ones128_np, sel2_np, par2_np, selt_np = _consts()

    i128b_dram = nc.inline_tensor(i128_np.astype(mybir.dt.np(BF16)), name="i128b")
    ibd2_dram = nc.inline_tensor(ibd2_np.astype(mybir.dt.np(BF16)), name="ibd2")
    iq2_dram = nc.inline_tensor(
        (3.25 * ibd2_np).astype(mybir.dt.np(BF16)), name="iq2"
    )
    ones_dram = nc.inline_tensor(ones128_np.astype(mybir.dt.np(BF16)), name="ones128")
    sel2_dram = nc.inline_tensor(sel2_np.astype(mybir.dt.np(BF16)), name="sel2")
    par2_dram = nc.inline_tensor(par2_np.astype(mybir.dt.np(BF16)), name="par2")
    selt_dram = nc.inline_tensor(selt_np.astype(mybir.dt.np(BF16)), name="selt")

    cpool = ctx.enter_context(tc.tile_pool(name="consts", bufs=1))
    qkpool = ctx.enter_context(tc.tile_pool(name="qk", bufs=3))
    vpool = ctx.enter_context(tc.tile_pool(name="v", bufs=3))
    tpool = ctx.enter_context(tc.tile_pool(name="trans", bufs=2))
    epool = ctx.enter_context(tc.tile_pool(name="exps", bufs=3))
    opool = ctx.enter_context(tc.tile_pool(name="outs", bufs=2))
    spool = ctx.enter_context(tc.tile_pool(name="smalls", bufs=4))
    ps_big = ctx.enter_context(tc.tile_pool(name="ps_big", bufs=2, space="PSUM"))
    ps_acc = ctx.enter_context(tc.tile_pool(name="ps_acc", bufs=1, space="PSUM"))
    ps_sm = ctx.enter_context(tc.tile_pool(name="ps_sm", bufs=2, space="PSUM"))

    i128b = cpool.tile([128, 128], BF16)
    nc.sync.dma_start(out=i128b[:], in_=i128b_dram[:])
    ibd2 = cpool.tile([128, 64], BF16)
    nc.sync.dma_start(out=ibd2[:], in_=ibd2_dram[:])
    iq2 = cpool.tile([128, 64], BF16)
    nc.sync.dma_start(out=iq2[:], in_=iq2_dram[:])
    ones128 = cpool.tile([128, 1], BF16)
    nc.sync.dma_start(out=ones128[:], in_=ones_dram[:])
    sel2 = cpool.tile([2, 128], BF16)
    nc.sync.dma_start(out=sel2[:], in_=sel2_dram[:])
    par16 = cpool.tile([128, 8], BF16)
    nc.sync.dma_start(out=par16[:], in_=par2_dram[:])
    selt = cpool.tile([128, 2], BF16)
    nc.sync.dma_start(out=selt[:], in_=selt_dram[:])

    for u in range(n_pairs // 2):
        pa, pb = 2 * u, 2 * u + 1

        # ---- loads ----
        qn = {}
        kn = {}
        for p, half in ((pa, 0), (pb, 1)):
            qn[half] = qkpool.tile([128, NT // 4, 4, 64], BF16, tag=f"qn{half}", name=f"qn{half}_{u}")
            nc.gpsimd.dma_start(
                out=qn[half][:].rearrange("p t q d -> p t (q d)"),
                in_=q_d[p].rearrange("(t p q) d -> p t (q d)", p=128, q=4),
            )
            kn[half] = qkpool.tile([128, NT // 4, 4, 64], BF16, tag=f"kn{half}", name=f"kn{half}_{u}")
            nc.gpsimd.dma_start(
                out=kn[half][:].rearrange("p t q d -> p t (q d)"),
                in_=k_d[p].rearrange("(t p q) d -> p t (q d)", p=128, q=4),
            )
        vb = {}
        for p, half in ((pa, 0), (pb, 1)):
            vb[half] = vpool.tile([128, NT // 4, 256], BF16, tag=f"vb{half}", name=f"vb{half}_{u}")
            nc.gpsimd.dma_start(
                out=vb[half][:],
                in_=v_d[p].rearrange("(t p q) d -> p t (q d)", p=128, q=4),
            )

        # ---- landmark pooling via column-window reduce ----
        qlm_ps = ps_acc.tile([128, 64], F32, tag="acc2")
        klm_ps = ps_acc.tile([128, 64], F32, tag="acc")
        for half in (0, 1):
            h = 64 * half
            for t in range(NT // 4):
                for j in range(4):
                    nc.tensor.matmul(
                        qlm_ps[h:h + 64, 8 * t:8 * t + 8],
                        qn[half][:, t, j, :],
                        par16[:],
                        start=(j == 0),
                        stop=(j == 3),
                    )
                    nc.tensor.matmul(
                        klm_ps[h:h + 64, 8 * t:8 * t + 8],
                        kn[half][:, t, j, :],
                        par16[:],
                        start=(j == 0),
                        stop=(j == 3),
                    )
        qlm2 = spool.tile([128, 64], BF16, tag="qlm2")
        nc.vector.tensor_copy(qlm2[:], qlm_ps[:])
        klm2 = spool.tile([128, 64], BF16, tag="klm2")
        nc.vector.tensor_copy(klm2[:], klm_ps[:])

        # ---- small scaled forms ----
        klmRAW = spool.tile([128, 64], BF16, tag="klmRAW")
        nc.vector.tensor_scalar_mul(klmRAW[:], klm2[:], CRAW)
        klmBD = spool.tile([128, 128], BF16, tag="klmBD")
        nc.vector.memset(klmBD[:], 0.0)
        nc.vector.tensor_scalar_mul(klmBD[0:64, 0:64], klm2[0:64, :], C1)
        nc.vector.tensor_scalar_mul(klmBD[64:128, 64:128], klm2[64:128, :], C1)
        qlmBD = spool.tile([128, 128], BF16, tag="qlmBD")
        nc.vector.memset(qlmBD[:], 0.0)
        nc.vector.tensor_scalar_mul(qlmBD[0:64, 0:64], qlm2[0:64, :], C1)
        nc.vector.tensor_scalar_mul(qlmBD[64:128, 64:128], qlm2[64:128, :], C1)

        # ---- kernel_2 = rownorm(exp(qlmBD.T @ klmRAW)), stacked [128, 64] ----
        l2_ps = ps_sm.tile([128, 64], F32, tag="sm")
        nc.tensor.matmul(l2_ps[:], qlmBD[:], klmRAW[:])
        e2 = spool.tile([128, 64], F32, tag="e2")
        d2 = spool.tile([128, 1], F32, tag="d2")
        nc.scalar.activation(e2[:], l2_ps[:], Exp, accum_out=d2[:])
        d2i = spool.tile([128, 1], F32, tag="d2i")
        nc.vector.reciprocal(d2i[:], d2[:])
        k2s = spool.tile([128, 64], BF16, tag="k2s")
        nc.vector.tensor_scalar_mul(k2s[:], e2[:], d2i[:])

        # k2^T per pair -> stacked [128, 64]
        k2t_ps = ps_sm.tile([128, 64], BF16, tag="sm")
        nc.tensor.transpose(k2t_ps[0:64, :], k2s[0:64, :], i128b[0:64, 0:64])
        nc.tensor.transpose(
            k2t_ps[64:128, :], k2s[64:128, :], i128b[64:128, 64:128]
        )
        k2ts = spool.tile([128, 64], BF16, tag="k2ts")
        nc.vector.tensor_copy(k2ts[:], k2t_ps[:])

        # ---- NS init: per-pair scale = 1/max(colsum) ----
        crow_ps = ps_sm.tile([2, 64], F32, tag="sm")
        nc.tensor.matmul(crow_ps[:], selt[:], k2s[:])
        mx = spool.tile([2, 1], F32, tag="mx")
        nc.vector.tensor_reduce(mx[:], crow_ps[:], axis=AX.X, op=Alu.max)
        scif = spool.tile([2, 1], F32, tag="scif")
        nc.vector.reciprocal(scif[:], mx[:])
        scit = spool.tile([2, 1], BF16, tag="scit")
        nc.vector.tensor_copy(scit[:], scif[:])
        scb_ps = ps_sm.tile([128, 1], F32, tag="sm")
        nc.tensor.matmul(scb_ps[:], sel2[:], scit[:])
        scb = spool.tile([128, 1], F32, tag="scb")
        nc.vector.tensor_copy(scb[:], scb_ps[:])

        vc = spool.tile([128, 64], BF16, tag="vc0")
        nc.vector.tensor_scalar_mul(vc[:], k2ts[:], scb[:])
        vct = spool.tile([128, 64], BF16, tag="vct0")
        nc.vector.tensor_scalar_mul(vct[:], k2s[:], scb[:])

        # ---- 6 Newton-Schulz iterations, per-pair matmuls + stacked smalls ----
        for i in range(6):
            a_ps = ps_sm.tile([128, 64], F32, tag="sm")
            at_ps = ps_sm.tile([128, 64], F32, tag="sm")
            for h in (0, 64):
                sl = slice(h, h + 64)
                nc.tensor.matmul(a_ps[sl, :], k2ts[sl, :], vc[sl, :])
                nc.tensor.matmul(at_ps[sl, :], vc[sl, :], k2ts[sl, :])
            at_sb = spool.tile([128, 64], BF16, tag="at_sb")
            nc.scalar.copy(at_sb[:], at_ps[:])
            b_sb = spool.tile([128, 64], BF16, tag="b_sb")
            nc.vector.scalar_tensor_tensor(
                b_sb[:], ibd2[:], 7.0, a_ps[:], op0=Alu.mult, op1=Alu.subtract
            )
            cc_ps = ps_sm.tile([128, 64], F32, tag="sm")
            for h in (0, 64):
                sl = slice(h, h + 64)
                nc.tensor.matmul(cc_ps[sl, :], at_sb[sl, :], b_sb[sl, :])
            d_sb = spool.tile([128, 64], BF16, tag="d_sb")
            nc.vector.scalar_tensor_tensor(
                d_sb[:], ibd2[:], 15.0, cc_ps[:], op0=Alu.mult, op1=Alu.subtract
            )
            f_ps = ps_sm.tile([128, 64], F32, tag="sm")
            for h in (0, 64):
                sl = slice(h, h + 64)
                nc.tensor.matmul(f_ps[sl, :], at_sb[sl, :], d_sb[sl, :])
            g_sb = spool.tile([128, 64], BF16, tag="g_sb")
            nc.vector.scalar_tensor_tensor(
                g_sb[:], f_ps[:], -0.25, iq2[:], op0=Alu.mult, op1=Alu.add
            )
            vnt_ps = ps_sm.tile([128, 64], F32, tag="sm")
            for h in (0, 64):
                sl = slice(h, h + 64)
                nc.tensor.matmul(vnt_ps[sl, :], g_sb[sl, :], vct[sl, :])
            if i < 5:
                vn_ps = ps_sm.tile([128, 64], F32, tag="sm")
                for h in (0, 64):
                    sl = slice(h, h + 64)
                    nc.tensor.matmul(vn_ps[sl, :], vct[sl, :], g_sb[sl, :])
                vc = spool.tile([128, 64], BF16, tag="vc", name=f"vc_{u}_{i}")
                nc.scalar.copy(vc[:], vn_ps[:])
            vct = spool.tile([128, 64], BF16, tag="vct", name=f"vct_{u}_{i}")
            nc.vector.tensor_copy(vct[:], vnt_ps[:])

        # ---- transposes: qt2/kt2 [128 = 2x64 d, S] bf16 ----
        qt2 = tpool.tile([128, S], BF16, tag="qt2")
        kt2 = tpool.tile([128, S], BF16, tag="kt2")
        for src, dst, ceng in ((qn, qt2, "act"), (kn, kt2, "dve")):
            for g in range(8):
                tp = ps_big.tile([128, 512], BF16, tag="bigb")
                for j in range(4):
                    t = 4 * g + j
                    for half in (0, 1):
                        nc.tensor.transpose(
                            tp[64 * half:64 * half + 64, 128 * j:128 * (j + 1)],
                            src[half][:, t // 4, t % 4, :],
                            i128b[:],
                        )
                if ceng == "act":
                    nc.scalar.copy(dst[:, 512 * g:512 * (g + 1)], tp[:])
                else:
                    nc.vector.tensor_copy(dst[:, 512 * g:512 * (g + 1)], tp[:])

        # ---- E1 stacked: e1t2 [128 = 2x64 m, S] ----
        e1t2 = epool.tile([128, S], BF16, tag="e1t2")
        for w in range(8):
            l1_ps = ps_big.tile([128, 512], F32, tag="big")
            nc.tensor.matmul(l1_ps[:], klmBD[:], qt2[:, 512 * w:512 * (w + 1)])
            nc.scalar.activation(e1t2[:, 512 * w:512 * (w + 1)], l1_ps[:], Exp)

        # ---- E3 + CV ----
        e3t = epool.tile([128, NT * 128], BF16, tag="e3t")
        for g in range(8):
            l3_ps = ps_big.tile([128, 512], F32, tag="big")
            for j in range(4):
                w = 4 * g + j
                nc.tensor.matmul(
                    l3_ps[:, 128 * j:128 * (j + 1)],
                    kt2[:, 128 * w:128 * (w + 1)],
                    qlmBD[:],
                )
            nc.scalar.activation(e3t[:, 512 * g:512 * (g + 1)], l3_ps[:], Exp)
        cv_ps = ps_acc.tile([128, 64], F32, tag="acc2")
        cvd_ps = ps_acc.tile([128, 1], F32, tag="acc")
        for half in (0, 1):
            h = 64 * half
            for t in range(NT):
                nc.tensor.matmul(
                    cvd_ps[h:h + 64, :],
                    e3t[:, 128 * t + 64 * half:128 * t + 64 * half + 64],
                    ones128[:],
                    start=(t == 0),
                    stop=(t == NT - 1),
                )
        for half in (0, 1):
            h = 64 * half
            for t in range(NT):
                nc.tensor.matmul(
                    cv_ps[h:h + 64, :],
                    e3t[:, 128 * t + 64 * half:128 * t + 64 * half + 64],
                    vb[half][:, t // 4, 64 * (t % 4):64 * (t % 4) + 64],
                    start=(t == 0),
                    stop=(t == NT - 1),
                )
        d3i = spool.tile([128, 1], F32, tag="d3i")
        nc.vector.reciprocal(d3i[:], cvd_ps[:])
        vctD = spool.tile([128, 64], BF16, tag="vctD")
        nc.vector.tensor_scalar_mul(vctD[:], vct[:], d3i[:])
        cv_sb = spool.tile([128, 64], BF16, tag="cv_sb")
        nc.vector.tensor_copy(cv_sb[:], cv_ps[:])

        # ---- M2' block-diag [128, 130] ----
        m2_ps = ps_sm.tile([128, 64], F32, tag="sm")
        for h in (0, 64):
            sl = slice(h, h + 64)
            nc.tensor.matmul(m2_ps[sl, :], vctD[sl, :], cv_sb[sl, :])
        m2bd = spool.tile([128, 130], BF16, tag="m2bd")
        nc.vector.memset(m2bd[:], 0.0)
        nc.scalar.copy(m2bd[0:64, 0:64], m2_ps[0:64, :])
        nc.scalar.copy(m2bd[64:128, 65:129], m2_ps[64:128, :])
        nc.vector.memset(m2bd[0:64, 64:65], 1.0)
        nc.vector.memset(m2bd[64:128, 129:130], 1.0)

        # ---- X' stacked via m2bd; normalize; store ----
        xsb = opool.tile([128, NT // 4, 2, 4, 64], F32, tag="xsb")
        for T in range(NT // 4):
            for qh in range(2):
                xpt = ps_big.tile([128, 512], F32, tag="big")
                xp = xpt[:, 0:260].rearrange("p (c e) -> p c e", e=130)
                for j in range(2):
                    w = 4 * T + 2 * qh + j
                    nc.tensor.matmul(
                        xp[:, j, :], e1t2[:, 128 * w:128 * (w + 1)], m2bd[:]
                    )
                xp4 = xp.rearrange("p c (q e) -> p c q e", q=2)
                dgi = spool.tile([128, 2, 2], F32, tag="dgi")
                nc.vector.reciprocal(dgi[:], xp4[:, :, :, 64])
                nc.vector.tensor_tensor(
                    xsb[:, T, :, 2 * qh:2 * qh + 2, :].rearrange(
                        "p r j d -> p j r d"
                    ),
                    xp4[:, :, :, 0:64],
                    dgi[:].rearrange("p c (q b) -> p c q b", b=1)
                    .broadcast_to([128, 2, 2, 64]),
                    op=Alu.mult,
                )
            if T in (3, NT // 4 - 1):
                lo = 0 if T == 3 else 4
                for p, half in ((pa, 0), (pb, 1)):
                    nc.sync.dma_start(
                        out=x_d[p].rearrange(
                            "(t p q) d -> p t (q d)", p=128, q=4
                        )[:, lo:T + 1, :],
                        in_=xsb[:, lo:T + 1, half, :, :].rearrange(
                            "p t q d -> p t (q d)"
                        ),
                    )


def build_nc(n_pairs=PAIRS, reps=1):
    from contextlib import ExitStack

    nc = bacc.Bacc("TRN2", target_bir_lowering=False, debug=False)
    q_d = nc.declare_dram_parameter("Q", [n_pairs, S, D], F32, isOutput=False)
    k_d = nc.declare_dram_parameter("K", [n_pairs, S, D], F32, isOutput=False)
    v_d = nc.declare_dram_parameter("V", [n_pairs, S, D], F32, isOutput=False)
    x_d = nc.declare_dram_parameter("X", [n_pairs, S, D], F32, isOutput=True)
    with tile.TileContext(nc) as tc:
        with ExitStack() as ctx:
            if reps == 1:
                build_body(tc, ctx, q_d[:], k_d[:], v_d[:], x_d[:], n_pairs)
            else:
                with tc.For_i(0, reps, 1):
                    build_body(tc, ctx, q_d[:], k_d[:], v_d[:], x_d[:], n_pairs)
    nc.finalize()
    return nc


_CACHED = {}


def kernel(Q: np.ndarray, K: np.ndarray, V: np.ndarray) -> np.ndarray:
    from concourse.bass_utils import run_bass_kernel_spmd

    if "nc" not in _CACHED:
        _CACHED["nc"] = build_nc()
    nc = _CACHED["nc"]

    qf = np.ascontiguousarray(Q.reshape(B * H, S, D), dtype=np.float32)
    kf = np.ascontiguousarray(K.reshape(B * H, S, D), dtype=np.float32)
    vf = np.ascontiguousarray(V.reshape(B * H, S, D), dtype=np.float32)
    core_ids = list(range(N_CORES))
    in_maps = [
        {
            "Q": qf[c * PAIRS:(c + 1) * PAIRS],
            "K": kf[c * PAIRS:(c + 1) * PAIRS],
            "V": vf[c * PAIRS:(c + 1) * PAIRS],
        }
        for c in core_ids
    ]
    res = run_bass_kernel_spmd(nc, in_maps, core_ids)
    out = np.concatenate([res.results[c]["X"] for c in core_ids], axis=0)
    return out.reshape(B, H, S, D)
